# revision 1
# baseline (speedup 1.0000x reference)
"""Bass/Trainium2 kernel for nn_BuildLstmUnrollNet.

Problem: 2-layer LSTM, unrolled T=11 steps with per-step (non-shared)
weights, B=8192, R=425, IN=20.  Output block t is the last-layer h
*before* step t, so only steps 0..9 need computing (step 10's weights
never affect the output).

Strategy (data-parallel over batch, 8 cores x 1024 rows):
  - States kept batch-major in ONE packed bf16 buffer per m-tile:
    cols [h0(425) | 1.0 | x(20) | h1(425) | pad(25)] = 896 = 7*128.
    Gates are computed batch-major in PSUM with the *transposed
    activations* stationary (lhsT) and the weights as the moving
    operand: layer 0 contracts over packed rows 0..511 (4 K-passes,
    bias + x folded in for free), layer 1 over rows 0..895 (7 K-passes
    -- h1 rides in the same packed buffer, so no ceil() waste).
  - Weights are host-prepacked+transposed to [K, 4R] bf16 blocks whose
    row layout matches the packed state buffer exactly.
  - The recurrent transpose h -> hT bounces through DRAM so the x-bar
    DMA transpose can do few, large [rows,128] -> [128,rows] blocks on
    the SP/HWDGE queue (no compute-engine cycles); the h0' chunks are
    transposed in 3 row-groups as the layer-0 cells complete so layer
    1's first batch tiles unblock early.
  - Cell math: ACT (one fused sigmoid over i|f|o + tanh straight out of
    PSUM), DVE muls/adds; c0/c1 stay fp32; h1 output written fp32.

kernel(**inputs) takes full-size numpy inputs, does the host-side
packing/sharding, runs the same program SPMD on cores 0..7, and
reassembles the full [8192, 4675] fp32 output (block 0 comes straight
from the initial state on the host).
"""

import numpy as np
import ml_dtypes

BF16 = ml_dtypes.bfloat16

B = 8192
NCORES = 8
BC = B // NCORES          # batch rows per core (1024)
NB = BC // 128            # m-tiles per core (8)
R = 425
IN = 20
GN = 4 * R                # 1700 gate columns
H1OFF = R + 1 + IN        # 446: h1 col offset in the packed state block
HC = 896                  # packed state block width (7*128)
NKC = HC // 128           # 7 transpose chunks
NK0 = 4                   # layer-0 K-passes (rows 0..511)
NK1 = 7                   # layer-1 K-passes (rows 0..895)
NKT = NK0 + NK1           # 11 weight K-blocks per step
NSTEPS = 10
# N chunks of the 1700-wide gate output (one PSUM bank each)
NCHUNKS = [(0, 512), (512, 512), (1024, 512), (1536, 164)]

# set by test.py to profile; results stashed in LAST_RESULT
TRACE = False
LAST_RESULT = None


def build_bass(n_steps=NSTEPS, finalize=True):
    import concourse.bacc as bacc
    import concourse.mybir as mybir
    import concourse.tile as tile

    f32 = mybir.dt.float32
    bf16 = mybir.dt.bfloat16
    Sig = mybir.ActivationFunctionType.Sigmoid
    Tanh = mybir.ActivationFunctionType.Tanh

    nc = bacc.Bacc()

    w_d = nc.declare_dram_parameter("w", [n_steps, 128, NKT * GN], bf16, False)
    hci_d = nc.declare_dram_parameter("hci", [128, NB * HC], bf16, False)
    htci_d = nc.declare_dram_parameter("htci", [128, NKC * BC], bf16, False)
    c0i_d = nc.declare_dram_parameter("c0i", [128, NB * R], f32, False)
    c1i_d = nc.declare_dram_parameter("c1i", [128, NB * R], f32, False)
    out_d = nc.declare_dram_parameter("out", [BC, n_steps * R], f32, True)
    # DRAM bounce buffer for the recurrent transpose (batch-major packed h)
    hd = nc.dram_tensor("hd", [BC, HC], bf16)

    with tile.TileContext(nc) as tc:
        with (
            tc.tile_pool(name="consts", bufs=1) as consts,
            tc.tile_pool(name="wpool", bufs=2) as wpool,
            tc.tile_pool(name="gpsum", bufs=2, space="PSUM") as gpsum,
            tc.tile_pool(name="tmp", bufs=3) as tmp,
        ):
            # persistent state tiles
            hs_t = consts.tile([128, NB * HC], bf16)   # packed batch-major
            htc = consts.tile([128, NKC * BC], bf16)   # transposed (lhsT)
            c0 = consts.tile([128, NB * R], f32)
            c1 = consts.tile([128, NB * R], f32)
            h1f = consts.tile([128, NB * R], f32)      # fp32 h1 for output

            # init DMAs on the SP (HWDGE) queue, most-urgent first, while
            # the first weight chunks stream on the Pool (SWDGE) queue
            for k in range(NKC):
                nc.sync.dma_start(htc[:, k * BC: (k + 1) * BC],
                                  htci_d[:, k * BC: (k + 1) * BC])
            nc.sync.dma_start(c0[:], c0i_d[:])
            nc.sync.dma_start(hs_t[:], hci_d[:])
            nc.sync.dma_start(c1[:], c1i_d[:])

            # step-0 weights, split per k-block so matmuls start early
            w = wpool.tile([128, NKT * GN], bf16, tag="w")
            for k in range(NKT):
                nc.gpsimd.dma_start(w[:, k * GN: (k + 1) * GN],
                                    w_d[0][:, k * GN: (k + 1) * GN])

            # PE warm-up: the HAM clock gate needs ~3.4us of sustained
            # activity before the PE runs at full rate.  Burn the initial
            # DMA wait with dummy matmuls on zeroed scratch so the ramp
            # clock starts before the real work does.
            warm = consts.tile([128, 128], bf16)
            nc.vector.memset(warm[:], 0.0)
            wps = gpsum.tile([128, 512], f32, tag="g")
            for i in range(20):
                nc.tensor.matmul(wps[:, 0: 128], warm[:], warm[:],
                                 start=True, stop=True)

            for t in range(n_steps):
                if t < n_steps - 1:
                    # next step's weights: few bulk chunks on the Pool queue
                    w_next = wpool.tile([128, NKT * GN], bf16, tag="w")
                    for c in range(4):
                        lo = c * 3 * GN
                        hi = min((c + 1) * 3 * GN, NKT * GN)
                        nc.gpsimd.dma_start(
                            w_next[:, lo: hi], w_d[t + 1][:, lo: hi])

                if t > 0:
                    # refresh the h1 rows (chunks 4..6) of the transposed
                    # state: h1^{t} was bounced to DRAM at the end of step
                    # t-1; layer 1 of this step reads it
                    for half in range(2):
                        rows = slice(half * 512, (half + 1) * 512)
                        for k in range(NK0, NKC):
                            nc.sync.dma_start(
                                out=htc[:, k * BC + half * 512:
                                        k * BC + (half + 1) * 512],
                                in_=hd[rows, k * 128: (k + 1) * 128],
                                transpose=True)

                for layer in range(2):
                    if layer == 0:
                        # (k-chunk of htc, W k-block)
                        kplan = [(k, k) for k in range(NK0)]
                    else:
                        # h1-only chunks (4..6) first: they are ready from
                        # the top-of-step transposes; the h0' chunks (0..3)
                        # are transposed mid-step after the layer-0 cells
                        kplan = ([(k, NK0 + k) for k in range(NK0, NKC)]
                                 + [(k, NK0 + k) for k in range(NK0)])
                    nk = len(kplan)
                    cst = c0 if layer == 0 else c1
                    for m in range(NB):
                        g = gpsum.tile([128, GN], f32, tag="g")
                        for ki, (kk, wk) in enumerate(kplan):
                            lhsT = htc[:, kk * BC + m * 128:
                                       kk * BC + (m + 1) * 128]
                            for (no, nw) in NCHUNKS:
                                nc.tensor.matmul(
                                    g[:, no: no + nw],
                                    lhsT,
                                    w[:, wk * GN + no: wk * GN + no + nw],
                                    start=(ki == 0),
                                    stop=(ki == nk - 1),
                                )

                        # LSTM cell (torch gate order: i, f, o, g).  One
                        # sigmoid over the contiguous i|f|o columns, one tanh.
                        cs = cst[:, m * R: (m + 1) * R]
                        tsig = tmp.tile([128, 3 * R], f32, tag="tsig")
                        nc.scalar.activation(tsig[:], g[:, 0: 3 * R], Sig)
                        ti = tsig[:, 0: R]
                        tf = tsig[:, R: 2 * R]
                        to = tsig[:, 2 * R: 3 * R]
                        tg = tmp.tile([128, R], f32, tag="tg")
                        nc.scalar.activation(tg[:], g[:, 3 * R: 4 * R], Tanh)

                        tig = tmp.tile([128, R], f32, tag="tig")
                        nc.vector.tensor_mul(tig[:], ti, tg[:])
                        tfc = tmp.tile([128, R], f32, tag="tfc")
                        nc.vector.tensor_mul(tfc[:], tf, cs)
                        nc.vector.tensor_add(cs, tfc[:], tig[:])
                        ttc = tmp.tile([128, R], f32, tag="ttc")
                        nc.scalar.activation(ttc[:], cs, Tanh)

                        # h writes + transposes ride the SP/HWDGE queue
                        # (cheap per-op); bulk W + out stores ride Pool/SWDGE
                        if layer == 0:
                            # h0_new -> packed bf16 cols 0..424, bounce the
                            # first 512 cols (incl ones/x consts and the
                            # still-current h1 rows 0..65) to DRAM
                            nc.vector.tensor_mul(
                                hs_t[:, m * HC: m * HC + R], to, ttc[:])
                            nc.sync.dma_start(
                                hd[m * 128: (m + 1) * 128, 0: 512],
                                hs_t[:, m * HC: m * HC + 512])
                        else:
                            hh = h1f[:, m * R: (m + 1) * R]
                            nc.vector.tensor_mul(hh, to, ttc[:])
                            nc.gpsimd.dma_start(
                                out_d[m * 128: (m + 1) * 128,
                                      t * R: (t + 1) * R], hh)
                            if t < n_steps - 1:
                                nc.vector.tensor_copy(
                                    hs_t[:, m * HC + H1OFF:
                                         m * HC + H1OFF + R], hh)
                                nc.sync.dma_start(
                                    hd[m * 128: (m + 1) * 128, 512: HC],
                                    hs_t[:, m * HC + 512: (m + 1) * HC])

                        # mid-step transpose of h0' chunk rows as soon as
                        # their m-tiles are written (3-way split: after m2,
                        # m5, m7) so layer 1's first M-tiles unblock early
                        if layer == 0 and m in (2, 5, 7):
                            lo = {2: 0, 5: 384, 7: 768}[m]
                            hi = {2: 384, 5: 768, 7: 1024}[m]
                            for k in range(NK0):
                                nc.sync.dma_start(
                                    out=htc[:, k * BC + lo: k * BC + hi],
                                    in_=hd[lo: hi, k * 128: (k + 1) * 128],
                                    transpose=True)
                if t < n_steps - 1:
                    w = w_next
    if finalize:
        nc.finalize()
    return nc


def _pack_pf(a):
    """[BC, C] -> [128, NB*C] with m-tile m at cols m*C."""
    c = a.shape[1]
    return np.ascontiguousarray(
        a.reshape(NB, 128, c).transpose(1, 0, 2).reshape(128, NB * c))


def _pack_kt(a):
    """[BC, HC] -> transposed [128, NKC*BC] with K-chunk k at cols k*BC."""
    return np.ascontiguousarray(
        a.T.reshape(NKC, 128, BC).transpose(1, 0, 2).reshape(128, NKC * BC))


def prep_inputs(x, init_states_input, W_i2h0, b_i2h0, W_h2h0, b_h2h0,
                W_i2h1, b_i2h1, W_h2h1, b_h2h1, n_steps=NSTEPS):
    """Host-side packing.  Returns (in_maps, h1_init_full)."""
    x = np.asarray(x, np.float32)
    init = np.asarray(init_states_input, np.float32)
    W_i2h0 = np.asarray(W_i2h0, np.float32)
    b_i2h0 = np.asarray(b_i2h0, np.float32)
    W_h2h0 = np.asarray(W_h2h0, np.float32)
    b_h2h0 = np.asarray(b_h2h0, np.float32)
    W_i2h1 = np.asarray(W_i2h1, np.float32)
    b_i2h1 = np.asarray(b_i2h1, np.float32)
    W_h2h1 = np.asarray(W_h2h1, np.float32)
    b_h2h1 = np.asarray(b_h2h1, np.float32)

    # per-step weight blocks, K-major, transposed to [K, 4R], rows
    # matching the packed state layout [h0 | 1 | x | h1 | pad]
    Wd = np.zeros((n_steps, NKT * 128, GN), np.float32)
    for t in range(n_steps):
        # layer-0 K-rows 0..511
        Wd[t, 0:R] = W_h2h0[t].T
        Wd[t, R] = b_i2h0[t] + b_h2h0[t]
        Wd[t, R + 1: R + 1 + IN] = W_i2h0[t].T
        # layer-1 K-rows 0..895 at block offset 4*128=512
        o = NK0 * 128
        Wd[t, o: o + R] = W_i2h1[t].T
        Wd[t, o + R] = b_i2h1[t] + b_h2h1[t]
        Wd[t, o + H1OFF: o + H1OFF + R] = W_h2h1[t].T
    w_dev = np.ascontiguousarray(
        Wd.reshape(n_steps, NKT, 128, GN).transpose(0, 2, 1, 3)
        .reshape(n_steps, 128, NKT * GN)).astype(BF16)

    init4 = init.reshape(B, 4, R)
    h0_full, c0_full = init4[:, 0], init4[:, 1]
    h1_full, c1_full = init4[:, 2], init4[:, 3]

    in_maps = []
    for c in range(NCORES):
        sl = slice(c * BC, (c + 1) * BC)
        hcp = np.zeros((BC, HC), np.float32)
        hcp[:, 0:R] = h0_full[sl]
        hcp[:, R] = 1.0
        hcp[:, R + 1: R + 1 + IN] = x[sl]
        hcp[:, H1OFF: H1OFF + R] = h1_full[sl]
        hcp = hcp.astype(BF16)
        in_maps.append({
            "w": w_dev,
            "hci": _pack_pf(hcp),
            "htci": _pack_kt(hcp),
            "c0i": _pack_pf(np.ascontiguousarray(c0_full[sl])),
            "c1i": _pack_pf(np.ascontiguousarray(c1_full[sl])),
        })
    return in_maps, h1_full


def kernel(x, init_states_input, W_i2h0, b_i2h0, W_h2h0, b_h2h0,
           W_i2h1, b_i2h1, W_h2h1, b_h2h1):
    global LAST_RESULT
    from concourse.bass_utils import run_bass_kernel_spmd

    in_maps, h1_full = prep_inputs(
        x, init_states_input, W_i2h0, b_i2h0, W_h2h0, b_h2h0,
        W_i2h1, b_i2h1, W_h2h1, b_h2h1)

    nc = build_bass(NSTEPS)
    res = run_bass_kernel_spmd(nc, in_maps, list(range(NCORES)), trace=TRACE)
    LAST_RESULT = res

    out = np.empty((B, (NSTEPS + 1) * R), np.float32)
    out[:, 0:R] = h1_full
    for c in range(NCORES):
        out[c * BC: (c + 1) * BC, R:] = res.results[c]["out"]
    return out



# revision 2
# speedup vs baseline: 1.2842x; 1.2842x over previous
"""Bass/Trainium2 kernel for nn_BuildLstmUnrollNet.

Problem: 2-layer LSTM, unrolled T=11 steps with per-step (non-shared)
weights, B=8192, R=425, IN=20.  Output block t is the last-layer h
*before* step t, so only steps 0..9 need computing.

Strategy (data-parallel over batch, 8 cores x 1024 rows):
  - Step 0 runs in bf16: its matmul operands are the *initial* states,
    which are unbounded N(0,1) draws -- fp8 there costs ~2.5e-2 rel err.
  - Steps 1..9 run the gate matmuls in fp8-e4m3 DoubleRow (both
    operands fp8, 256-deep contraction per pass): after step 0 every h
    is a tanh*sigmoid product bounded by 1, and e4m3 keeps the end-to-
    end rel err ~4e-3 (vs the 2e-2 gate).  DoubleRow halves both the
    pass count and the per-column cost.
  - States kept batch-major in ONE packed bf16 buffer per m-tile:
    cols [h0(425) | 1.0 | x(20) | h1(425) | 1.0 | pad] = 896 = 7*128.
    The second 1.0 (col 871) pairs with a host-precomputed fp8 residual
    weight row that cancels most of layer-1's bias quantization error.
  - Gates are computed batch-major in PSUM with transposed activations
    stationary (lhsT) and weights moving: layer 0 contracts packed rows
    0..511, layer 1 rows 0..1023 (chunk 7 is a zeroed pad chunk so
    layer 1 is exactly 4 DoubleRow passes).
  - Weights host-prepacked+transposed to [K, 4R] blocks matching the
    packed state rows: bf16 table for step 0, fp8 tables (12 blocks
    incl. zero block 11) for steps 1..9.
  - The recurrent transpose h -> hT bounces through DRAM so the x-bar
    DMA transpose runs on the SP/HWDGE queue; fp8 can't ride the 2-byte
    transpose path, so the GPSIMD/Pool engine (otherwise idle) converts
    the transposed bf16 chunks to fp8 right after each transpose group.
  - Cell math all-bf16 (2x DVE mode): one fused sigmoid over i|f|o,
    tanh(g), two muls + add for c, tanh(c), final mul written straight
    into the packed state; the output DMA reads the packed h1 slice
    (out tensor is bf16, upconverted on the host).

kernel(**inputs) takes full-size numpy inputs, packs/shards on the
host, runs the same program SPMD on cores 0..7, and reassembles the
full [8192, 4675] fp32 output (block 0 comes from the initial state).
"""

import numpy as np
import ml_dtypes

BF16 = ml_dtypes.bfloat16
FP8 = ml_dtypes.float8_e4m3

B = 8192
NCORES = 8
BC = B // NCORES          # batch rows per core (1024)
NB = BC // 128            # m-tiles per core (8)
R = 425
IN = 20
GN = 4 * R                # 1700 gate columns
H1OFF = R + 1 + IN        # 446: h1 col offset in the packed state block
HC = 896                  # packed state block width (7*128)
NKC = 7                   # bf16 transpose chunks
NK0 = 4                   # layer-0 K-chunks (rows 0..511)
NKT = 11                  # step-0 bf16 weight K-blocks (4 + 7)
NW8 = 12                  # fp8 weight K-blocks (4 + 8, block 11 zero)
NSTEPS = 10
# N chunks of the 1700-wide gate output (one PSUM bank each)
NCHUNKS = [(0, 512), (512, 512), (1024, 512), (1536, 164)]

# set by test.py to profile; results stashed in LAST_RESULT
TRACE = False
LAST_RESULT = None


def build_bass(n_steps=NSTEPS, finalize=True):
    import concourse.bacc as bacc
    import concourse.mybir as mybir
    import concourse.tile as tile

    f32 = mybir.dt.float32
    bf16 = mybir.dt.bfloat16
    fp8 = mybir.dt.float8e4
    Sig = mybir.ActivationFunctionType.Sigmoid
    Tanh = mybir.ActivationFunctionType.Tanh
    DR = mybir.MatmulPerfMode.DoubleRow

    nc = bacc.Bacc()

    n8 = max(n_steps - 1, 1)
    w0_d = nc.declare_dram_parameter("w0", [128, NKT * GN], bf16, False)
    w8_d = nc.declare_dram_parameter("w8", [n8, 128, NW8 * GN], fp8, False)
    hci_d = nc.declare_dram_parameter("hci", [128, NB * HC], bf16, False)
    htci_d = nc.declare_dram_parameter("htci", [128, NKC * BC], bf16, False)
    c0i_d = nc.declare_dram_parameter("c0i", [128, NB * R], bf16, False)
    c1i_d = nc.declare_dram_parameter("c1i", [128, NB * R], bf16, False)
    out_d = nc.declare_dram_parameter("out", [BC, n_steps * R], bf16, True)
    # DRAM bounce buffer for the recurrent transpose (batch-major packed h)
    hd = nc.dram_tensor("hd", [BC, HC], bf16)

    with tile.TileContext(nc) as tc:
        with (
            tc.tile_pool(name="consts", bufs=1) as consts,
            tc.tile_pool(name="wpool", bufs=2) as wpool,
            tc.tile_pool(name="gpsum", bufs=2, space="PSUM") as gpsum,
            tc.tile_pool(name="tmp", bufs=3) as tmp,
        ):
            # persistent state tiles
            hs_t = consts.tile([128, NB * HC], bf16)    # packed batch-major
            htc = consts.tile([128, NKC, BC], bf16)     # transposed (lhsT)
            htc8 = consts.tile([128, 8, BC], fp8)       # fp8 lhsT (DR)
            c0 = consts.tile([128, NB * R], bf16)
            c1 = consts.tile([128, NB * R], bf16)
            w0t = consts.tile([128, NKT * GN], bf16)    # step-0 weights

            # init DMAs on the SP (HWDGE) queue, most-urgent first, while
            # the weight tables stream on the Pool (SWDGE) queue
            for k in range(NKC):
                nc.sync.dma_start(htc[:, k, :], htci_d[:, k * BC:(k + 1) * BC])
            nc.sync.dma_start(c0[:], c0i_d[:])
            nc.sync.dma_start(hs_t[:], hci_d[:])
            nc.sync.dma_start(c1[:], c1i_d[:])

            # step-0 weights, split per k-block so matmuls start early
            for k in range(NKT):
                nc.gpsimd.dma_start(w0t[:, k * GN:(k + 1) * GN],
                                    w0_d[:, k * GN:(k + 1) * GN])
            # first fp8 table (for step 1)
            w8t = None
            if n_steps > 1:
                w8t = wpool.tile([128, NW8, GN], fp8, tag="w8")
                for c in range(4):
                    nc.gpsimd.dma_start(w8t[:, 3 * c:3 * (c + 1), :],
                                        w8_d[0][:, 3 * c * GN:3 * (c + 1) * GN])
            # zero pad chunk for layer-1's 4th DoubleRow pass
            nc.gpsimd.memset(htc8[:, 7, :], 0.0)

            # PE warm-up: keep the p-state ramp busy while init DMAs land
            warm = consts.tile([128, 128], bf16)
            nc.vector.memset(warm[:], 0.0)
            wps = gpsum.tile([128, 512], f32, tag="g")
            for i in range(20):
                nc.tensor.matmul(wps[:, 0:128], warm[:], warm[:],
                                 start=True, stop=True)

            def cell(g, layer, m, t):
                """LSTM cell from PSUM gates g (torch order i,f,o,g).
                All-bf16 elementwise; h written into the packed state."""
                cst = c0 if layer == 0 else c1
                cs = cst[:, m * R:(m + 1) * R]
                tsig = tmp.tile([128, 3 * R], bf16, tag="tsig")
                nc.scalar.activation(tsig[:], g[:, 0:3 * R], Sig)
                tg = tmp.tile([128, R], bf16, tag="tg")
                nc.scalar.activation(tg[:], g[:, 3 * R:4 * R], Tanh)
                tig = tmp.tile([128, R], bf16, tag="tig")
                nc.vector.tensor_mul(tig[:], tsig[:, 0:R], tg[:])
                tfc = tmp.tile([128, R], bf16, tag="tfc")
                nc.vector.tensor_mul(tfc[:], tsig[:, R:2 * R], cs)
                nc.vector.tensor_add(cs, tfc[:], tig[:])
                ttc = tmp.tile([128, R], bf16, tag="ttc")
                nc.scalar.activation(ttc[:], cs, Tanh)
                off = m * HC + (0 if layer == 0 else H1OFF)
                nc.vector.tensor_mul(hs_t[:, off:off + R],
                                     tsig[:, 2 * R:3 * R], ttc[:])

            def l0_post(m, t):
                """Bounce h0' (+ consts) to DRAM; transpose+convert in 3
                row-groups so layer 1's first m-tiles unblock early."""
                nc.sync.dma_start(hd[m * 128:(m + 1) * 128, 0:512],
                                  hs_t[:, m * HC:m * HC + 512])
                if m in (2, 5, 7):
                    lo = {2: 0, 5: 384, 7: 768}[m]
                    hi = {2: 384, 5: 768, 7: 1024}[m]
                    for k in range(NK0):
                        nc.sync.dma_start(out=htc[:, k, lo:hi],
                                          in_=hd[lo:hi, k * 128:(k + 1) * 128],
                                          transpose=True)
                    nc.gpsimd.tensor_copy(htc8[:, 0:NK0, lo:hi],
                                          htc[:, 0:NK0, lo:hi])

            def l1_post(m, t):
                # h1 (+ consts incl. the residual-ones col) to DRAM
                nc.gpsimd.dma_start(
                    out_d[m * 128:(m + 1) * 128, t * R:(t + 1) * R],
                    hs_t[:, m * HC + H1OFF:m * HC + H1OFF + R])
                if t < n_steps - 1:
                    nc.sync.dma_start(hd[m * 128:(m + 1) * 128, 512:HC],
                                      hs_t[:, m * HC + 512:(m + 1) * HC])

            # ---- step 0: bf16 ----
            for layer in range(2):
                if layer == 0:
                    kplan = [(k, k) for k in range(NK0)]
                else:
                    kplan = ([(k, NK0 + k) for k in range(NK0, NKC)]
                             + [(k, NK0 + k) for k in range(NK0)])
                nk = len(kplan)
                for m in range(NB):
                    g = gpsum.tile([128, GN], f32, tag="g")
                    for ki, (kk, wk) in enumerate(kplan):
                        lhsT = htc[:, kk, m * 128:(m + 1) * 128]
                        for (no, nw) in NCHUNKS:
                            nc.tensor.matmul(
                                g[:, no:no + nw], lhsT,
                                w0t[:, wk * GN + no:wk * GN + no + nw],
                                start=(ki == 0), stop=(ki == nk - 1))
                    cell(g, layer, m, 0)
                    if layer == 0:
                        l0_post(m, 0)
                    else:
                        l1_post(m, 0)

            # ---- steps 1..n-1: fp8 DoubleRow ----
            for t in range(1, n_steps):
                if t < n_steps - 1:
                    w8_next = wpool.tile([128, NW8, GN], fp8, tag="w8")
                    for c in range(4):
                        nc.gpsimd.dma_start(
                            w8_next[:, 3 * c:3 * (c + 1), :],
                            w8_d[t][:, 3 * c * GN:3 * (c + 1) * GN])

                # refresh h1 rows: transposed from last step's bounce, then
                # converted to fp8 on the Pool engine
                for half in range(2):
                    rows = slice(half * 512, (half + 1) * 512)
                    for k in range(NK0, NKC):
                        nc.sync.dma_start(
                            out=htc[:, k, half * 512:(half + 1) * 512],
                            in_=hd[rows, k * 128:(k + 1) * 128],
                            transpose=True)
                    nc.gpsimd.tensor_copy(
                        htc8[:, NK0:NKC, half * 512:(half + 1) * 512],
                        htc[:, NK0:NKC, half * 512:(half + 1) * 512])

                for layer in range(2):
                    if layer == 0:
                        # DR pass j: state chunks (2j,2j+1) x w blocks same
                        jplan = [(0, 0), (1, 1)]
                    else:
                        # h1 chunk-pairs first (ready at top of step), then
                        # the h0' pairs (transposed mid-step)
                        jplan = [(2, 2), (3, 3), (0, 0), (1, 1)]
                    nj = len(jplan)
                    for m in range(NB):
                        g = gpsum.tile([128, GN], f32, tag="g")
                        for ji, (sj, wj) in enumerate(jplan):
                            wb = 2 * wj + (0 if layer == 0 else 4)
                            lhsT = htc8[:, 2 * sj:2 * sj + 2,
                                        m * 128:(m + 1) * 128]
                            for (no, nw) in NCHUNKS:
                                nc.tensor.matmul(
                                    g[:, no:no + nw], lhsT,
                                    w8t[:, wb:wb + 2, no:no + nw],
                                    start=(ji == 0), stop=(ji == nj - 1),
                                    perf_mode=DR)
                        cell(g, layer, m, t)
                        if layer == 0:
                            l0_post(m, t)
                        else:
                            l1_post(m, t)
                if t < n_steps - 1:
                    w8t = w8_next
    if finalize:
        nc.finalize()
    return nc


def _pack_pf(a):
    """[BC, C] -> [128, NB*C] with m-tile m at cols m*C."""
    c = a.shape[1]
    return np.ascontiguousarray(
        a.reshape(NB, 128, c).transpose(1, 0, 2).reshape(128, NB * c))


def _pack_kt(a):
    """[BC, HC] -> transposed [128, NKC*BC] with K-chunk k at cols k*BC."""
    return np.ascontiguousarray(
        a.T.reshape(NKC, 128, BC).transpose(1, 0, 2).reshape(128, NKC * BC))


def prep_inputs(x, init_states_input, W_i2h0, b_i2h0, W_h2h0, b_h2h0,
                W_i2h1, b_i2h1, W_h2h1, b_h2h1, n_steps=NSTEPS):
    """Host-side packing.  Returns (in_maps, h1_init_full)."""
    x = np.asarray(x, np.float32)
    init = np.asarray(init_states_input, np.float32)
    W_i2h0 = np.asarray(W_i2h0, np.float32)
    b_i2h0 = np.asarray(b_i2h0, np.float32)
    W_h2h0 = np.asarray(W_h2h0, np.float32)
    b_h2h0 = np.asarray(b_h2h0, np.float32)
    W_i2h1 = np.asarray(W_i2h1, np.float32)
    b_i2h1 = np.asarray(b_i2h1, np.float32)
    W_h2h1 = np.asarray(W_h2h1, np.float32)
    b_h2h1 = np.asarray(b_h2h1, np.float32)

    # step-0 bf16 weight table: K-major blocks, transposed to [K, 4R],
    # rows matching the packed state layout [h0 | 1 | x | h1 | 1 | pad]
    Wd0 = np.zeros((NKT * 128, GN), np.float32)
    Wd0[0:R] = W_h2h0[0].T
    Wd0[R] = b_i2h0[0] + b_h2h0[0]
    Wd0[R + 1:R + 1 + IN] = W_i2h0[0].T
    o = NK0 * 128
    Wd0[o:o + R] = W_i2h1[0].T
    Wd0[o + R] = b_i2h1[0] + b_h2h1[0]
    Wd0[o + H1OFF:o + H1OFF + R] = W_h2h1[0].T
    w0_dev = np.ascontiguousarray(
        Wd0.reshape(NKT, 128, GN).transpose(1, 0, 2)
        .reshape(128, NKT * GN)).astype(BF16)

    # fp8 tables for steps 1..n-1: blocks 0..3 layer 0 (512 rows),
    # blocks 4..11 layer 1 (1024 rows incl. zero pad + bias residual)
    n8 = max(n_steps - 1, 1)
    Wd8 = np.zeros((n8, NW8 * 128, GN), np.float32)
    for t in range(1, n_steps):
        d = Wd8[t - 1]
        d[0:R] = W_h2h0[t].T
        d[R] = b_i2h0[t] + b_h2h0[t]
        d[R + 1:R + 1 + IN] = W_i2h0[t].T
        o = NK0 * 128
        d[o:o + R] = W_i2h1[t].T
        b1 = b_i2h1[t] + b_h2h1[t]
        d[o + R] = b1
        d[o + H1OFF:o + H1OFF + R] = W_h2h1[t].T
        # residual row (pairs with the 1.0 at packed col 871): cancels
        # most of the fp8 quantization error of the layer-1 bias row
        d[o + H1OFF + R] = b1 - b1.astype(FP8).astype(np.float32)
    w8_dev = np.ascontiguousarray(
        Wd8.reshape(n8, NW8, 128, GN).transpose(0, 2, 1, 3)
        .reshape(n8, 128, NW8 * GN)).astype(FP8)

    init4 = init.reshape(B, 4, R)
    h0_full, c0_full = init4[:, 0], init4[:, 1]
    h1_full, c1_full = init4[:, 2], init4[:, 3]

    in_maps = []
    for c in range(NCORES):
        sl = slice(c * BC, (c + 1) * BC)
        hcp = np.zeros((BC, HC), np.float32)
        hcp[:, 0:R] = h0_full[sl]
        hcp[:, R] = 1.0
        hcp[:, R + 1:R + 1 + IN] = x[sl]
        hcp[:, H1OFF:H1OFF + R] = h1_full[sl]
        hcp[:, H1OFF + R] = 1.0
        hcp = hcp.astype(BF16)
        in_maps.append({
            "w0": w0_dev,
            "w8": w8_dev,
            "hci": _pack_pf(hcp),
            "htci": _pack_kt(hcp),
            "c0i": _pack_pf(np.ascontiguousarray(c0_full[sl])).astype(BF16),
            "c1i": _pack_pf(np.ascontiguousarray(c1_full[sl])).astype(BF16),
        })
    return in_maps, h1_full


def kernel(x, init_states_input, W_i2h0, b_i2h0, W_h2h0, b_h2h0,
           W_i2h1, b_i2h1, W_h2h1, b_h2h1):
    global LAST_RESULT
    from concourse.bass_utils import run_bass_kernel_spmd

    in_maps, h1_full = prep_inputs(
        x, init_states_input, W_i2h0, b_i2h0, W_h2h0, b_h2h0,
        W_i2h1, b_i2h1, W_h2h1, b_h2h1)

    nc = build_bass(NSTEPS)
    res = run_bass_kernel_spmd(nc, in_maps, list(range(NCORES)), trace=TRACE)
    LAST_RESULT = res

    out = np.empty((B, (NSTEPS + 1) * R), np.float32)
    out[:, 0:R] = h1_full
    for c in range(NCORES):
        out[c * BC:(c + 1) * BC, R:] = res.results[c]["out"].astype(np.float32)
    return out
